# revision 18
# baseline (speedup 1.0000x reference)
"""Cross-attention Trainium2 kernel (8 NeuronCores, SPMD).

Reference computation (B=4, N_q=1024, N_kv=4096, DIM=1024, H=16, hd=64):
    k = (kv @ W_qkv[:, 1024:2048]), v = (kv @ W_qkv[:, 2048:3072])
    qh = (q @ W_qkv[:, 0:1024])
    out = softmax(qh k^T / 8) v @ W_proj + b_proj

Sharding: core c -> (batch c//2, head-group c%2 of 8 heads).  All on-chip
tensors are feature-major (host pre-transposes q/kv and casts to bf16).
Attention runs in the S^T = [kv, q] layout so no on-chip transposes are
needed anywhere:
  - K^T tile [128 = 2 heads' dims, kv] is the direct GEMM output of
    W_k-stationary x kv^T.
  - S^T chunk [128 kv, q] = lhsT(K^T slice, K=64) x rhs(Q^T) with the two
    heads of a pair row-packed into PE row-groups 0-63 / 64-127.
  - exp fused into PSUM eviction on ScalarE (scale=1/8), bf16 out.
  - x^T = lhsT(V augmented with a ones column, M=65) x rhs(S'^T); PSUM row
    64 accumulates the softmax denominators for free.
  - head-partial output projection (contraction over this core's 512 head
    dims) + ReduceScatter(add) over the core pair completes the projection
    and leaves each core its own 512 q rows; bias (zeroed on odd cores) is
    added by a K=1 ones-row matmul.

Phase A runs for two pairs at a time ("super-group") so the V projection
streams N=256 columns per matmul, keeping LDWEIGHTS hidden, and kv^T is
DMA-streamed once per group.  Attention for group g overlaps phase A of
group g+1 through Tile's dependency scheduling.
"""

import numpy as np

import concourse.bacc as bacc
import concourse.bass as bass
import concourse.mybir as mybir
import concourse.tile as tile
from concourse.bass_utils import run_bass_kernel_spmd

DT16 = mybir.dt.float16
F32 = mybir.dt.float32
AF = mybir.ActivationFunctionType

P = 128
DIM = 1024
NKV = 4096
NQ = 1024
NQH = 512  # q rows produced per core
SCALE = 0.125

_CACHE = {}


def build(sim=False, reps=1, no_cc=False):
    key = ("sim" if sim else "nc") + (f"_r{reps}" if reps != 1 else "") + ("_nocc" if no_cc else "")
    if key in _CACHE:
        return _CACHE[key]

    nc = bacc.Bacc("TRN2", target_bir_lowering=False, debug=False,
                   num_devices=1 if sim else 8)

    kvT_e = nc.dram_tensor("kvT", [DIM, NKV], DT16, kind="ExternalInput")
    qT_e = nc.dram_tensor("qT", [DIM, NQ], DT16, kind="ExternalInput")
    wq_e = nc.dram_tensor("wq", [DIM, 512], DT16, kind="ExternalInput")
    wk_e = nc.dram_tensor("wk", [DIM, 512], DT16, kind="ExternalInput")
    wv_e = nc.dram_tensor("wv", [DIM, 512], DT16, kind="ExternalInput")
    wp_e = nc.dram_tensor("wp", [DIM, 512], DT16, kind="ExternalInput")
    bias_e = nc.dram_tensor("bias", [1, 512], DT16, kind="ExternalInput")
    out_e = nc.dram_tensor("out", [NQ, 512], F32, kind="ExternalOutput")

    with tile.TileContext(nc) as tc:
        with (
            tc.tile_pool(name="const", bufs=1) as constp,
            tc.tile_pool(name="big", bufs=1) as bigp,
            tc.tile_pool(name="kvs", bufs=2) as kvsp,
            tc.tile_pool(name="qs", bufs=3) as qsp,
            tc.tile_pool(name="ep", bufs=3) as ep,
            tc.tile_pool(name="ev", bufs=2) as evp,
            tc.tile_pool(name="nrm", bufs=1) as nrmp,
            tc.tile_pool(name="ps", bufs=1, space="PSUM") as psp,
            tc.tile_pool(name="dram", bufs=1, space="DRAM") as dramp,
        ):
            # ---- resident weights (gpsimd DMA queues: keep the sync queue
            # free for the latency-critical kv/q streams) ----
            wq_sb = constp.tile([P, 8 * 512], DT16, tag="wq")
            wk_sb = constp.tile([P, 8 * 512], DT16, tag="wk")
            wv_sb = constp.tile([P, 8 * 512], DT16, tag="wv")
            wp_sb = constp.tile([P, 8 * 512], DT16, tag="wp")
            bias_sb = constp.tile([1, DIM], DT16, tag="bias")
            ones_sb = constp.tile([1, P], DT16, tag="ones")
            for d in range(8):
                nc.gpsimd.dma_start(wk_sb[:, 512 * d:512 * (d + 1)],
                                    wk_e[P * d:P * (d + 1), :])
                nc.gpsimd.dma_start(wv_sb[:, 512 * d:512 * (d + 1)],
                                    wv_e[P * d:P * (d + 1), :])
                nc.gpsimd.dma_start(wq_sb[:, 512 * d:512 * (d + 1)],
                                    wq_e[P * d:P * (d + 1), :])
            for d in range(8):
                nc.gpsimd.dma_start(wp_sb[:, 512 * d:512 * (d + 1)],
                                    wp_e[P * d:P * (d + 1), :])
            nc.gpsimd.dma_start(bias_sb[:, 0:512], bias_e[:])
            nc.vector.memset(ones_sb[:], 1.0)

            # ---- persistent per-pair tensors ----
            KT = [bigp.tile([P, NKV], DT16, tag=f"kt{p}", name=f"kt{p}")
                  for p in range(4)]
            QT = [bigp.tile([P, NQ], DT16, tag=f"qt{p}", name=f"qt{p}")
                  for p in range(4)]
            V = [bigp.tile([P, 32 * 130], DT16, tag=f"v{p}", name=f"v{p}")
                 for p in range(4)]
            XT = [bigp.tile([P, NQ], DT16, tag=f"xt{p}", name=f"xt{p}")
                  for p in range(4)]
            v_views = [V[p].rearrange("p (k o) -> p k o", o=130) for p in range(4)]

            for rep in range(reps):
                for p in range(4):
                    # ones columns of augmented V: cols 64/129 of each 130-block
                    nc.vector.memset(v_views[p][:, :, 64:65], 1.0)
                    nc.vector.memset(v_views[p][:, :, 129:130], 1.0)

                for g in range(2):
                    pair_list = [2 * g, 2 * g + 1]

                    # ---------- phase A: K^T and V projections ----------
                    for r in range(8):  # kv ranges of 512
                        kvr = []
                        for d in range(8):
                            t = kvsp.tile([P, 512], DT16, tag=f"kv{d}",
                                          name=f"kv{d}")
                            nc.sync.dma_start(
                                t[:],
                                kvT_e[P * d:P * (d + 1), 512 * r:512 * (r + 1)])
                            kvr.append(t)
                        for p in pair_list:
                            ps_kt = psp.tile([P, 512], F32, tag="pp", bufs=2,
                                             name="pskt")
                            for d in range(8):
                                nc.tensor.matmul(
                                    ps_kt[:],
                                    wk_sb[:, 512 * d + P * p:512 * d + P * (p + 1)],
                                    kvr[d][:],
                                    start=(d == 0), stop=(d == 7))
                            nc.vector.tensor_copy(
                                KT[p][:, 512 * r:512 * (r + 1)], ps_kt[:])
                        # V for both pairs of the group: N=256 per matmul
                        for k in range(4):
                            ps_v = psp.tile([P, 512], F32, tag="pp", bufs=2,
                                            name="psv")
                            for d in range(8):
                                nc.tensor.matmul(
                                    ps_v[:, 0:256],
                                    kvr[d][:, P * k:P * (k + 1)],
                                    wv_sb[:, 512 * d + 256 * g:512 * d + 256 * (g + 1)],
                                    start=(d == 0), stop=(d == 7))
                            for j, p in enumerate(pair_list):
                                src = ps_v[:, P * j:P * (j + 1)].rearrange(
                                    "p (h e) -> p h e", e=64)
                                dst = v_views[p][:, 4 * r + k, :].rearrange(
                                    "p (h e) -> p h e", e=65)[:, :, 0:64]
                                nc.vector.tensor_copy(dst, src)

                    # ---------- Q^T for both pairs ----------
                    qts = []
                    for d in range(8):
                        qt = qsp.tile([P, NQ], DT16, tag="qd", name="qd")
                        nc.sync.dma_start(qt[:], qT_e[P * d:P * (d + 1), :])
                        qts.append(qt)
                    for p in pair_list:
                        ps_q = psp.tile([P, NQ], F32, tag="s", bufs=2, name="psq")
                        for d in range(8):
                            for n in range(2):
                                nc.tensor.matmul(
                                    ps_q[:, 512 * n:512 * (n + 1)],
                                    wq_sb[:, 512 * d + P * p:512 * d + P * (p + 1)],
                                    qts[d][:, 512 * n:512 * (n + 1)],
                                    start=(d == 0), stop=(d == 7))
                        nc.vector.tensor_copy(QT[p][:], ps_q[:])

                    # ---------- phase B: attention ----------
                    for p in pair_list:
                        for qn in range(2):
                            xA = psp.tile([65, 512], F32, tag="xA", bufs=1,
                                          name="xA")
                            xB = psp.tile([65, 512], F32, tag="xB", bufs=1,
                                          name="xB")
                            for i in range(32):
                                s = psp.tile([P, NQ], F32, tag="s", bufs=2,
                                             name="s")
                                nc.tensor.matmul(
                                    s[:, 0:512],
                                    KT[p][0:64, P * i:P * (i + 1)],
                                    QT[p][0:64, 512 * qn:512 * (qn + 1)],
                                    start=True, stop=True)
                                nc.tensor.matmul(
                                    s[:, 512:1024],
                                    KT[p][64:128, P * i:P * (i + 1)],
                                    QT[p][64:128, 512 * qn:512 * (qn + 1)],
                                    start=True, stop=True)
                                e = ep.tile([P, NQ], DT16, tag="e", name="e")
                                nc.scalar.activation(e[:], s[:], AF.Exp,
                                                     scale=SCALE)
                                nc.tensor.matmul(
                                    xA[:], V[p][:, 130 * i:130 * i + 65],
                                    e[:, 0:512],
                                    start=(i == 0), stop=(i == 31))
                                nc.tensor.matmul(
                                    xB[:], V[p][:, 130 * i + 65:130 * (i + 1)],
                                    e[:, 512:1024],
                                    start=(i == 0), stop=(i == 31))
                            # normalize by accumulated denominators (row 64)
                            rA = nrmp.tile([1, 512], F32, tag="rA", name="rA")
                            rB = nrmp.tile([1, 512], F32, tag="rB", name="rB")
                            nc.vector.reciprocal(rA[:], xA[64:65, :])
                            nc.vector.reciprocal(rB[:], xB[64:65, :])
                            bA = nrmp.tile([64, 512], F32, tag="bA", name="bA")
                            bB = nrmp.tile([64, 512], F32, tag="bB", name="bB")
                            nc.gpsimd.partition_broadcast(bA[:], rA[:])
                            nc.gpsimd.partition_broadcast(bB[:], rB[:])
                            nc.vector.tensor_mul(
                                XT[p][0:64, 512 * qn:512 * (qn + 1)],
                                xA[0:64, :], bA[:])
                            xbt = evp.tile([64, 512], DT16, tag="xbt",
                                           name="xbt")
                            nc.vector.tensor_mul(xbt[:], xB[0:64, :], bB[:])
                            # head B lives on partitions 64-127: sbuf->sbuf DMA
                            nc.sync.dma_start(
                                XT[p][64:128, 512 * qn:512 * (qn + 1)], xbt[:])

                # ---------- phase C: AllGather x^T + column-split projection ----------
                xt_dram = dramp.tile([NQH, NQ], DT16, name="xt_dram")
                for p in range(4):
                    nc.sync.dma_start(xt_dram[P * p:P * (p + 1), :], XT[p][:])
                xg_dram = dramp.tile([NQ, NQ], DT16, name="xg_dram")
                nc.gpsimd.collective_compute(
                    "AllGather",
                    mybir.AluOpType.bypass,
                    replica_groups=[[0, 1], [2, 3], [4, 5], [6, 7]],
                    ins=[xt_dram.opt()],
                    outs=[xg_dram.opt()],
                )
                XF = [bigp.tile([P, NQ], DT16, tag=f"xf{d}", name=f"xf{d}")
                      for d in range(8)]
                for d in range(8):
                    nc.sync.dma_start(XF[d][:], xg_dram[P * d:P * (d + 1), :])
                for m in range(8):
                    po = psp.tile([P, 512], F32, tag="pp", bufs=2, name="po")
                    for d in range(8):
                        nc.tensor.matmul(
                            po[:],
                            XF[d][:, P * m:P * (m + 1)],
                            wp_sb[:, 512 * d:512 * (d + 1)],
                            start=(d == 0), stop=False)
                    nc.tensor.matmul(
                        po[:], ones_sb[0:1, :], bias_sb[0:1, 0:512],
                        start=False, stop=True)
                    ob = evp.tile([P, 512], F32, tag="ob", name="ob")
                    nc.vector.tensor_copy(ob[:], po[:])
                    nc.sync.dma_start(out_e[P * m:P * (m + 1), :], ob[:])

    nc.compile()
    _CACHE[key] = nc
    return nc


def _prep_in_maps(q, kv, W_qkv, W_proj, b_proj):
    bf = np.float16
    q = np.asarray(q, dtype=np.float32)
    kv = np.asarray(kv, dtype=np.float32)
    W_qkv = np.asarray(W_qkv, dtype=np.float32)
    W_proj = np.asarray(W_proj, dtype=np.float32)
    b_proj = np.asarray(b_proj, dtype=np.float32)

    wp = np.ascontiguousarray(W_proj).astype(bf)
    bias = np.ascontiguousarray(b_proj.reshape(1, DIM)).astype(bf)
    kvT = [np.ascontiguousarray(kv[b].T).astype(bf) for b in range(4)]
    qT = [np.ascontiguousarray(q[b].T).astype(bf) for b in range(4)]

    in_maps = []
    for c in range(8):
        b, hg = c // 2, c % 2
        in_maps.append({
            "kvT": kvT[b],
            "qT": qT[b],
            "wq": np.ascontiguousarray(
                W_qkv[:, 512 * hg:512 * (hg + 1)]).astype(bf),
            "wk": np.ascontiguousarray(
                W_qkv[:, 1024 + 512 * hg:1024 + 512 * (hg + 1)]).astype(bf),
            "wv": np.ascontiguousarray(
                W_qkv[:, 2048 + 512 * hg:2048 + 512 * (hg + 1)]).astype(bf),
            "wp": np.ascontiguousarray(wp[:, 512 * hg:512 * (hg + 1)]),
            "bias": np.ascontiguousarray(bias[:, 512 * hg:512 * (hg + 1)]),
        })
    return in_maps


def run(q, kv, W_qkv, W_proj, b_proj, trace=False):
    nc = build()
    in_maps = _prep_in_maps(q, kv, W_qkv, W_proj, b_proj)
    res = run_bass_kernel_spmd(nc, in_maps, list(range(8)), trace=trace)
    out = np.zeros((4, NQ, DIM), np.float32)
    for c in range(8):
        b, hg = c // 2, c % 2
        out[b, :, 512 * hg:512 * (hg + 1)] = res.results[c]["out"]
    return out, res


def kernel(q, kv, W_qkv, W_proj, b_proj):
    out, _ = run(q, kv, W_qkv, W_proj, b_proj)
    return out


# revision 19
# speedup vs baseline: 1.1991x; 1.1991x over previous
"""Cross-attention Trainium2 kernel (8 NeuronCores, SPMD, collective-free).

Reference computation (B=4, N_q=1024, N_kv=4096, DIM=1024, H=16, hd=64):
    k = (kv @ W_qkv[:, 1024:2048]), v = (kv @ W_qkv[:, 2048:3072])
    qh = (q @ W_qkv[:, 0:1024])
    out = softmax(qh k^T / 8) v @ W_proj + b_proj

Sharding: core c -> (batch c//2, q-half c%2).  Each core computes ALL 16
heads for its 512 queries, so the output projection contracts the full
1024 dims locally and no cross-core collective is needed (a pair
ReduceScatter measured ~150-200us, far above wire rate).  The K/V
projections are duplicated across the two cores of a batch (+55us of PE,
mostly hidden under the exp-bound attention phase).

Everything on-chip is fp16 feature-major (host pre-transposes and casts;
fp16 matmuls run at full 1 cycle/row and fp16 keeps input rounding at
2^-11).  Attention runs in the S^T = [kv, q] layout so no on-chip
transposes exist anywhere:
  - K^T pair-tile [128 = 2 heads' dims, kv] is the direct GEMM output of
    W_k-stationary x kv^T.
  - S^T chunk [128 kv, 512 q] per head; the pair's two heads are
    row-packed into PE row-groups 0-63 / 64-127 (K=64 each, concurrent).
  - exp is fused into PSUM eviction on ScalarE (scale=1/8), fp16 out.
  - PV is col-packed: head A -> PSUM rows 0-63 (col-group 0), head B ->
    rows 64-127 (col-group 64), concurrent M=64 matmuls.  The shared bank
    is memset once and all PV matmuls use start=False so the two
    interleaved accumulation groups cannot clear each other.
  - softmax denominators: VectorE keeps a running fp16 sum of the exp
    tiles; one GpSimd partition_all_reduce per pair sums across kv rows
    and broadcasts, then an approx-reciprocal feeds the normalization
    multiplies (the per-partition fp16 rounding averages out ~11x in the
    128-partition reduction).
  - kv is processed in two 2048 halves with K^T/V tiles reused, keeping
    SBUF under budget; sums/x partials accumulate across halves.
"""

import numpy as np

import concourse.bacc as bacc
import concourse.bass as bass
import concourse.bass_isa as bass_isa
import concourse.mybir as mybir
import concourse.tile as tile
from concourse.bass_utils import run_bass_kernel_spmd

DT16 = mybir.dt.float16
F32 = mybir.dt.float32
AF = mybir.ActivationFunctionType

P = 128
DIM = 1024
NKV = 4096
NQ = 1024
NQH = 512   # q rows per core
HKV = 2048  # kv half processed per phase-A/B round
SCALE = 0.125

_CACHE = {}


def build(sim=False, reps=1, no_cc=False):
    key = ("sim" if sim else "nc") + (f"_r{reps}" if reps != 1 else "")
    if key in _CACHE:
        return _CACHE[key]

    nc = bacc.Bacc("TRN2", target_bir_lowering=False, debug=False,
                   num_devices=1 if sim else 8)

    kvT_e = nc.dram_tensor("kvT", [DIM, NKV], DT16, kind="ExternalInput")
    qT_e = nc.dram_tensor("qT", [DIM, NQH], DT16, kind="ExternalInput")
    wq_e = nc.dram_tensor("wq", [DIM, DIM], DT16, kind="ExternalInput")
    wk_e = nc.dram_tensor("wk", [DIM, DIM], DT16, kind="ExternalInput")
    wv_e = nc.dram_tensor("wv", [DIM, DIM], DT16, kind="ExternalInput")
    wp_e = nc.dram_tensor("wp", [DIM, DIM], DT16, kind="ExternalInput")
    bias_e = nc.dram_tensor("bias", [1, DIM], DT16, kind="ExternalInput")
    out_e = nc.dram_tensor("out", [NQH, DIM], F32, kind="ExternalOutput")

    with tile.TileContext(nc) as tc:
        with (
            tc.tile_pool(name="const", bufs=1) as constp,
            tc.tile_pool(name="big", bufs=1) as bigp,
            tc.tile_pool(name="kvs", bufs=2) as kvsp,
            tc.tile_pool(name="qs", bufs=2) as qsp,
            tc.tile_pool(name="ep", bufs=3) as ep,
            tc.tile_pool(name="ev", bufs=2) as evp,
            tc.tile_pool(name="nrm", bufs=1) as nrmp,
            tc.tile_pool(name="ps", bufs=1, space="PSUM") as psp,
        ):
            # ---- resident weights (gpsimd DMA queues: keep the sync queue
            # free for the latency-critical kv/q streams) ----
            wk_sb = constp.tile([P, 8 * DIM], DT16, tag="wk")
            wv_sb = constp.tile([P, 8 * DIM], DT16, tag="wv")
            wp_sb = constp.tile([P, 8 * DIM], DT16, tag="wp")
            bias_sb = constp.tile([1, DIM], DT16, tag="bias")
            ones_sb = constp.tile([1, P], DT16, tag="ones")
            for d in range(8):
                nc.gpsimd.dma_start(wk_sb[:, DIM * d:DIM * (d + 1)],
                                    wk_e[P * d:P * (d + 1), :])
                nc.gpsimd.dma_start(wv_sb[:, DIM * d:DIM * (d + 1)],
                                    wv_e[P * d:P * (d + 1), :])
                nc.gpsimd.dma_start(wp_sb[:, DIM * d:DIM * (d + 1)],
                                    wp_e[P * d:P * (d + 1), :])
            nc.gpsimd.dma_start(bias_sb[:], bias_e[:])
            nc.vector.memset(ones_sb[:], 1.0)

            # ---- persistent per-pair tensors (8 pairs = 16 heads) ----
            KT = [bigp.tile([P, HKV], DT16, tag=f"kt{p}", name=f"kt{p}")
                  for p in range(8)]
            V = [bigp.tile([P, 16 * P], DT16, tag=f"v{p}", name=f"v{p}")
                 for p in range(8)]
            QT = [bigp.tile([P, NQH], DT16, tag=f"qt{p}", name=f"qt{p}")
                  for p in range(8)]
            # x partials (normalized in place at the end -> proj lhsT)
            XP = [bigp.tile([P, NQH], DT16, tag=f"xp{p}", name=f"xp{p}")
                  for p in range(8)]
            # running softmax denominators, fp16, accumulated across halves
            SAC = [bigp.tile([P, NQ], DT16, tag=f"sac{p}", name=f"sac{p}")
                   for p in range(8)]
            v_views = [V[p].rearrange("p (k o) -> p k o", o=P) for p in range(8)]

            for rep in range(reps):
                # ---------- Q^T projection (q and wq streamed, d-outer) ----------
                for pset in range(2):
                    ps_qs = [psp.tile([P, NQH], F32,
                                      tag=("s" if j < 2 else "x2"),
                                      bufs=2, name=f"qps{j}")
                             for j in range(4)]
                    for d in range(8):
                        qt = qsp.tile([P, NQH], DT16, tag="qd", name="qd")
                        nc.sync.dma_start(qt[:], qT_e[P * d:P * (d + 1), :])
                        w = qsp.tile([P, NQH], DT16, tag="wqd", name="wqd")
                        nc.sync.dma_start(
                            w[:], wq_e[P * d:P * (d + 1),
                                       NQH * pset:NQH * (pset + 1)])
                        for pp_ in range(4):
                            nc.tensor.matmul(
                                ps_qs[pp_][:],
                                w[:, P * pp_:P * (pp_ + 1)],
                                qt[:],
                                start=(d == 0), stop=(d == 7))
                    for pp_ in range(4):
                        nc.vector.tensor_copy(QT[4 * pset + pp_][:],
                                              ps_qs[pp_][:])

                for half in range(2):
                    kv0 = HKV * half
                    # ---------- phase A(half): K^T and V for all pairs ----------
                    for r in range(4):  # 512-ranges within the half
                        kvr = []
                        for d in range(8):
                            t = kvsp.tile([P, 512], DT16, tag=f"kv{d}",
                                          name=f"kv{d}")
                            nc.sync.dma_start(
                                t[:], kvT_e[P * d:P * (d + 1),
                                            kv0 + 512 * r:kv0 + 512 * (r + 1)])
                            kvr.append(t)
                        for p in range(8):
                            ps_kt = psp.tile([P, 512], F32, tag="pp", bufs=2,
                                             name="pskt")
                            for d in range(8):
                                nc.tensor.matmul(
                                    ps_kt[:],
                                    wk_sb[:, DIM * d + P * p:DIM * d + P * (p + 1)],
                                    kvr[d][:],
                                    start=(d == 0), stop=(d == 7))
                            nc.vector.tensor_copy(
                                KT[p][:, 512 * r:512 * (r + 1)], ps_kt[:])
                        # V: N=512 per matmul (pairs 4v..4v+3 per v-group)
                        for k in range(4):
                            for v in range(2):
                                ps_v = psp.tile([P, 512], F32, tag="pp",
                                                bufs=2, name="psv")
                                for d in range(8):
                                    nc.tensor.matmul(
                                        ps_v[:],
                                        kvr[d][:, P * k:P * (k + 1)],
                                        wv_sb[:, DIM * d + 512 * v:
                                              DIM * d + 512 * (v + 1)],
                                        start=(d == 0), stop=(d == 7))
                                for j in range(4):
                                    p = 4 * v + j
                                    nc.vector.tensor_copy(
                                        v_views[p][:, 4 * r + k, :],
                                        ps_v[:, P * j:P * (j + 1)])

                    # ---------- phase B(half): attention ----------
                    for p in range(8):
                        x2 = psp.tile([P, NQH], F32, tag="x2", bufs=2,
                                      name="x2")
                        # two interleaved col-group accumulations share this
                        # bank: memset + start=False keeps them independent
                        nc.vector.memset(x2[:], 0.0)
                        for i in range(16):
                            s = psp.tile([P, NQ], F32, tag="s", bufs=2,
                                         name="s")
                            nc.tensor.matmul(
                                s[:, 0:512],
                                KT[p][0:64, P * i:P * (i + 1)],
                                QT[p][0:64, :],
                                start=True, stop=True)
                            nc.tensor.matmul(
                                s[:, 512:1024],
                                KT[p][64:128, P * i:P * (i + 1)],
                                QT[p][64:128, :],
                                start=True, stop=True)
                            e = ep.tile([P, NQ], DT16, tag="e", name="e")
                            nc.scalar.activation(e[:], s[:], AF.Exp,
                                                 scale=SCALE)
                            nc.tensor.matmul(
                                x2[0:64, :], V[p][:, P * i:P * i + 64],
                                e[:, 0:512], tile_position=(0, 0),
                                start=False, stop=(i == 15))
                            nc.tensor.matmul(
                                x2[64:128, :], V[p][:, P * i + 64:P * (i + 1)],
                                e[:, 512:1024], tile_position=(0, 64),
                                start=False, stop=(i == 15))
                            if half == 0 and i == 0:
                                nc.vector.tensor_copy(SAC[p][:], e[:])
                            else:
                                nc.vector.tensor_add(SAC[p][:], SAC[p][:], e[:])
                        if half == 0:
                            nc.vector.tensor_copy(XP[p][:], x2[:])
                        else:
                            nc.vector.tensor_add(XP[p][:], XP[p][:], x2[:])

                # ---------- normalize + output projection ----------
                for p in range(8):
                    rsum = nrmp.tile([P, NQ], F32, tag="rsum", bufs=1,
                                     name="rsum")
                    nc.gpsimd.partition_all_reduce(
                        rsum[:], SAC[p][:], channels=P,
                        reduce_op=bass_isa.ReduceOp.add)
                    rinv = nrmp.tile([P, NQ], F32, tag="rinv", bufs=1,
                                     name="rinv")
                    nc.vector.reciprocal_approx_fast(rinv[:], rsum[:])
                    nc.vector.tensor_mul(XP[p][0:64, :], XP[p][0:64, :],
                                         rinv[0:64, 0:512])
                    nc.vector.tensor_mul(XP[p][64:128, :], XP[p][64:128, :],
                                         rinv[64:128, 512:1024])

                for m in range(4):
                    for n in range(2):
                        po = psp.tile([P, 512], F32, tag="pp", bufs=2,
                                      name="po")
                        for d in range(8):
                            nc.tensor.matmul(
                                po[:],
                                XP[d][:, P * m:P * (m + 1)],
                                wp_sb[:, DIM * d + 512 * n:DIM * d + 512 * (n + 1)],
                                start=(d == 0), stop=False)
                        nc.tensor.matmul(
                            po[:], ones_sb[0:1, :],
                            bias_sb[0:1, 512 * n:512 * (n + 1)],
                            start=False, stop=True)
                        ob = evp.tile([P, 512], F32, tag="ob", name="ob")
                        nc.vector.tensor_copy(ob[:], po[:])
                        nc.sync.dma_start(
                            out_e[P * m:P * (m + 1), 512 * n:512 * (n + 1)],
                            ob[:])

    nc.compile()
    _CACHE[key] = nc
    return nc


def _prep_in_maps(q, kv, W_qkv, W_proj, b_proj):
    f16 = np.float16
    q = np.asarray(q, dtype=np.float32)
    kv = np.asarray(kv, dtype=np.float32)
    W_qkv = np.asarray(W_qkv, dtype=np.float32)
    W_proj = np.asarray(W_proj, dtype=np.float32)
    b_proj = np.asarray(b_proj, dtype=np.float32)

    wq = np.ascontiguousarray(W_qkv[:, 0:DIM]).astype(f16)
    wk = np.ascontiguousarray(W_qkv[:, DIM:2 * DIM]).astype(f16)
    wv = np.ascontiguousarray(W_qkv[:, 2 * DIM:3 * DIM]).astype(f16)
    wp = np.ascontiguousarray(W_proj).astype(f16)
    bias = np.ascontiguousarray(b_proj.reshape(1, DIM)).astype(f16)
    kvT = [np.ascontiguousarray(kv[b].T).astype(f16) for b in range(4)]
    qT = [np.ascontiguousarray(q[b].T).astype(f16) for b in range(4)]

    in_maps = []
    for c in range(8):
        b, qh = c // 2, c % 2
        in_maps.append({
            "kvT": kvT[b],
            "qT": np.ascontiguousarray(qT[b][:, NQH * qh:NQH * (qh + 1)]),
            "wq": wq, "wk": wk, "wv": wv, "wp": wp,
            "bias": bias,
        })
    return in_maps


def run(q, kv, W_qkv, W_proj, b_proj, trace=False):
    nc = build()
    in_maps = _prep_in_maps(q, kv, W_qkv, W_proj, b_proj)
    res = run_bass_kernel_spmd(nc, in_maps, list(range(8)), trace=trace)
    out = np.zeros((4, NQ, DIM), np.float32)
    for c in range(8):
        b, qh = c // 2, c % 2
        out[b, NQH * qh:NQH * (qh + 1), :] = res.results[c]["out"]
    return out, res


def kernel(q, kv, W_qkv, W_proj, b_proj):
    out, _ = run(q, kv, W_qkv, W_proj, b_proj)
    return out


# revision 21
# speedup vs baseline: 1.6818x; 1.4025x over previous
"""Cross-attention Trainium2 kernel (8 NeuronCores, SPMD, collective-free).

Reference computation (B=4, N_q=1024, N_kv=4096, DIM=1024, H=16, hd=64):
    k = (kv @ W_qkv[:, 1024:2048]), v = (kv @ W_qkv[:, 2048:3072])
    qh = (q @ W_qkv[:, 0:1024])
    out = softmax(qh k^T / 8) v @ W_proj + b_proj

Sharding: core c -> (batch c//2, q-half c%2).  Each core computes ALL 16
heads for its 512 queries, so the output projection contracts the full
1024 dims locally and no cross-core collective is needed (a pair
ReduceScatter measured ~150-200us, far above wire rate).  The K/V
projections are duplicated across the two cores of a batch (+55us of PE,
mostly hidden under the exp-bound attention phase).

Everything on-chip is fp16 feature-major (host pre-transposes and casts;
fp16 matmuls run at full 1 cycle/row and fp16 keeps input rounding at
2^-11).  Attention runs in the S^T = [kv, q] layout so no on-chip
transposes exist anywhere:
  - K^T pair-tile [128 = 2 heads' dims, kv] is the direct GEMM output of
    W_k-stationary x kv^T.
  - S^T chunk [128 kv, 512 q] per head; the pair's two heads are
    row-packed into PE row-groups 0-63 / 64-127 (K=64 each, concurrent).
  - exp is fused into PSUM eviction on ScalarE (scale=1/8), fp16 out.
  - PV is col-packed: head A -> PSUM rows 0-63 (col-group 0), head B ->
    rows 64-127 (col-group 64), concurrent M=64 matmuls.  The shared bank
    is memset once and all PV matmuls use start=False so the two
    interleaved accumulation groups cannot clear each other.
  - softmax denominators: VectorE keeps a running fp16 sum of the exp
    tiles; one GpSimd partition_all_reduce per pair sums across kv rows
    and broadcasts, then an approx-reciprocal feeds the normalization
    multiplies (the per-partition fp16 rounding averages out ~11x in the
    128-partition reduction).
  - kv is processed in two 2048 halves with K^T/V tiles reused, keeping
    SBUF under budget; sums/x partials accumulate across halves.
"""

import numpy as np

import concourse.bacc as bacc
import concourse.bass as bass
import concourse.bass_isa as bass_isa
import concourse.mybir as mybir
import concourse.tile as tile
from concourse.bass_utils import run_bass_kernel_spmd

DT16 = mybir.dt.float16
F32 = mybir.dt.float32
AF = mybir.ActivationFunctionType

P = 128
DIM = 1024
NKV = 4096
NQ = 1024
NQH = 512   # q rows per core
HKV = 2048  # kv half processed per phase-A/B round
SCALE = 0.125

_CACHE = {}


def build(sim=False, reps=1, no_cc=False):
    key = ("sim" if sim else "nc") + (f"_r{reps}" if reps != 1 else "")
    if key in _CACHE:
        return _CACHE[key]

    nc = bacc.Bacc("TRN2", target_bir_lowering=False, debug=False,
                   num_devices=1 if sim else 8)

    kvT_e = nc.dram_tensor("kvT", [DIM, NKV], DT16, kind="ExternalInput")
    qT_e = nc.dram_tensor("qT", [DIM, NQH], DT16, kind="ExternalInput")
    wq_e = nc.dram_tensor("wq", [DIM, DIM], DT16, kind="ExternalInput")
    wk_e = nc.dram_tensor("wk", [DIM, DIM], DT16, kind="ExternalInput")
    wv_e = nc.dram_tensor("wv", [DIM, DIM], DT16, kind="ExternalInput")
    wp_e = nc.dram_tensor("wp", [DIM, DIM], DT16, kind="ExternalInput")
    bias_e = nc.dram_tensor("bias", [1, DIM], DT16, kind="ExternalInput")
    out_e = nc.dram_tensor("out", [NQH, DIM], F32, kind="ExternalOutput")

    with tile.TileContext(nc) as tc:
        with (
            tc.tile_pool(name="const", bufs=1) as constp,
            tc.tile_pool(name="big", bufs=1) as bigp,
            tc.tile_pool(name="kvs", bufs=2) as kvsp,
            tc.tile_pool(name="qs", bufs=2) as qsp,
            tc.tile_pool(name="ep", bufs=3) as ep,
            tc.tile_pool(name="ev", bufs=2) as evp,
            tc.tile_pool(name="nrm", bufs=1) as nrmp,
            tc.tile_pool(name="ps", bufs=1, space="PSUM") as psp,
        ):
            # ---- resident weights (gpsimd DMA queues: keep the sync queue
            # free for the latency-critical kv/q streams) ----
            wk_sb = constp.tile([P, 8 * DIM], DT16, tag="wk")
            wv_sb = constp.tile([P, 8 * DIM], DT16, tag="wv")
            wp_sb = constp.tile([P, 8 * DIM], DT16, tag="wp")
            bias_sb = constp.tile([1, DIM], DT16, tag="bias")
            ones_sb = constp.tile([1, P], DT16, tag="ones")
            for d in range(8):
                nc.gpsimd.dma_start(wk_sb[:, DIM * d:DIM * (d + 1)],
                                    wk_e[P * d:P * (d + 1), :])
                nc.gpsimd.dma_start(wv_sb[:, DIM * d:DIM * (d + 1)],
                                    wv_e[P * d:P * (d + 1), :])
                nc.gpsimd.dma_start(wp_sb[:, DIM * d:DIM * (d + 1)],
                                    wp_e[P * d:P * (d + 1), :])
            nc.gpsimd.dma_start(bias_sb[:], bias_e[:])
            nc.vector.memset(ones_sb[:], 1.0)

            # ---- persistent per-pair tensors (8 pairs = 16 heads) ----
            KT = [bigp.tile([P, HKV], DT16, tag=f"kt{p}", name=f"kt{p}")
                  for p in range(8)]
            V = [bigp.tile([P, 16 * P], DT16, tag=f"v{p}", name=f"v{p}")
                 for p in range(8)]
            QT = [bigp.tile([P, NQH], DT16, tag=f"qt{p}", name=f"qt{p}")
                  for p in range(8)]
            # x partials (normalized in place at the end -> proj lhsT)
            XP = [bigp.tile([P, NQH], DT16, tag=f"xp{p}", name=f"xp{p}")
                  for p in range(8)]
            # running softmax denominators, fp16, accumulated across halves
            SAC = [bigp.tile([P, NQ], DT16, tag=f"sac{p}", name=f"sac{p}")
                   for p in range(8)]
            v_views = [V[p].rearrange("p (k o) -> p k o", o=P) for p in range(8)]

            for rep in range(reps):
                # ---------- Q^T projection (q and wq streamed, d-outer) ----------
                for pset in range(2):
                    ps_qs = [psp.tile([P, NQH], F32,
                                      tag=("s" if j < 2 else
                                           ("x2a" if j == 2 else "x2b")),
                                      bufs=(2 if j < 2 else 1),
                                      name=f"qps{j}")
                             for j in range(4)]
                    for d in range(8):
                        qt = qsp.tile([P, NQH], DT16, tag="qd", name="qd")
                        nc.sync.dma_start(qt[:], qT_e[P * d:P * (d + 1), :])
                        w = qsp.tile([P, NQH], DT16, tag="wqd", name="wqd")
                        nc.sync.dma_start(
                            w[:], wq_e[P * d:P * (d + 1),
                                       NQH * pset:NQH * (pset + 1)])
                        for pp_ in range(4):
                            nc.tensor.matmul(
                                ps_qs[pp_][:],
                                w[:, P * pp_:P * (pp_ + 1)],
                                qt[:],
                                start=(d == 0), stop=(d == 7))
                    for pp_ in range(4):
                        nc.vector.tensor_copy(QT[4 * pset + pp_][:],
                                              ps_qs[pp_][:])

                for half in range(2):
                    kv0 = HKV * half
                    # ---------- phase A(half): K^T and V for all pairs ----------
                    for r in range(4):  # 512-ranges within the half
                        kvr = []
                        for d in range(8):
                            t = kvsp.tile([P, 512], DT16, tag=f"kv{d}",
                                          name=f"kv{d}")
                            nc.sync.dma_start(
                                t[:], kvT_e[P * d:P * (d + 1),
                                            kv0 + 512 * r:kv0 + 512 * (r + 1)])
                            kvr.append(t)
                        for p in range(8):
                            ps_kt = psp.tile([P, 512], F32, tag="pp", bufs=2,
                                             name="pskt")
                            for d in range(8):
                                nc.tensor.matmul(
                                    ps_kt[:],
                                    wk_sb[:, DIM * d + P * p:DIM * d + P * (p + 1)],
                                    kvr[d][:],
                                    start=(d == 0), stop=(d == 7))
                            nc.vector.tensor_copy(
                                KT[p][:, 512 * r:512 * (r + 1)], ps_kt[:])
                        # V: N=512 per matmul (pairs 4v..4v+3 per v-group)
                        for k in range(4):
                            for v in range(2):
                                ps_v = psp.tile([P, 512], F32, tag="pp",
                                                bufs=2, name="psv")
                                for d in range(8):
                                    nc.tensor.matmul(
                                        ps_v[:],
                                        kvr[d][:, P * k:P * (k + 1)],
                                        wv_sb[:, DIM * d + 512 * v:
                                              DIM * d + 512 * (v + 1)],
                                        start=(d == 0), stop=(d == 7))
                                for j in range(4):
                                    p = 4 * v + j
                                    nc.vector.tensor_copy(
                                        v_views[p][:, 4 * r + k, :],
                                        ps_v[:, P * j:P * (j + 1)])

                    # ---------- phase B(half): attention ----------
                    for p in range(8):
                        # one PSUM bank per head: independent accumulation
                        # groups with normal start/stop flags, no shared-bank
                        # serialization
                        x2a = psp.tile([P, NQH], F32, tag="x2a", bufs=1,
                                       name="x2a")
                        x2b = psp.tile([P, NQH], F32, tag="x2b", bufs=1,
                                       name="x2b")
                        for i in range(16):
                            s = psp.tile([P, NQ], F32, tag="s", bufs=2,
                                         name="s")
                            nc.tensor.matmul(
                                s[:, 0:512],
                                KT[p][0:64, P * i:P * (i + 1)],
                                QT[p][0:64, :],
                                start=True, stop=True)
                            nc.tensor.matmul(
                                s[:, 512:1024],
                                KT[p][64:128, P * i:P * (i + 1)],
                                QT[p][64:128, :],
                                start=True, stop=True)
                            e = ep.tile([P, NQ], DT16, tag="e", name="e")
                            nc.scalar.activation(e[:], s[:], AF.Exp,
                                                 scale=SCALE)
                            nc.tensor.matmul(
                                x2a[0:64, :], V[p][:, P * i:P * i + 64],
                                e[:, 0:512], tile_position=(0, 0),
                                start=(i == 0), stop=(i == 15))
                            nc.tensor.matmul(
                                x2b[64:128, :], V[p][:, P * i + 64:P * (i + 1)],
                                e[:, 512:1024], tile_position=(0, 64),
                                start=(i == 0), stop=(i == 15))
                            if half == 0 and i == 0:
                                nc.vector.tensor_copy(SAC[p][:], e[:])
                            else:
                                nc.vector.tensor_add(SAC[p][:], SAC[p][:], e[:])
                        if half == 0:
                            nc.vector.tensor_copy(XP[p][0:64, :], x2a[0:64, :])
                            nc.vector.tensor_copy(XP[p][64:128, :],
                                                  x2b[64:128, :])
                        else:
                            nc.vector.tensor_add(XP[p][0:64, :],
                                                 XP[p][0:64, :], x2a[0:64, :])
                            nc.vector.tensor_add(XP[p][64:128, :],
                                                 XP[p][64:128, :],
                                                 x2b[64:128, :])

                # ---------- normalize + output projection ----------
                for p in range(8):
                    rsum = nrmp.tile([P, NQ], F32, tag="rsum", bufs=1,
                                     name="rsum")
                    nc.gpsimd.partition_all_reduce(
                        rsum[:], SAC[p][:], channels=P,
                        reduce_op=bass_isa.ReduceOp.add)
                    rinv = nrmp.tile([P, NQ], F32, tag="rinv", bufs=1,
                                     name="rinv")
                    nc.vector.reciprocal_approx_fast(rinv[:], rsum[:])
                    nc.vector.tensor_mul(XP[p][0:64, :], XP[p][0:64, :],
                                         rinv[0:64, 0:512])
                    nc.vector.tensor_mul(XP[p][64:128, :], XP[p][64:128, :],
                                         rinv[64:128, 512:1024])

                for m in range(4):
                    for n in range(2):
                        po = psp.tile([P, 512], F32, tag="pp", bufs=2,
                                      name="po")
                        for d in range(8):
                            nc.tensor.matmul(
                                po[:],
                                XP[d][:, P * m:P * (m + 1)],
                                wp_sb[:, DIM * d + 512 * n:DIM * d + 512 * (n + 1)],
                                start=(d == 0), stop=False)
                        nc.tensor.matmul(
                            po[:], ones_sb[0:1, :],
                            bias_sb[0:1, 512 * n:512 * (n + 1)],
                            start=False, stop=True)
                        ob = evp.tile([P, 512], F32, tag="ob", name="ob")
                        nc.vector.tensor_copy(ob[:], po[:])
                        nc.sync.dma_start(
                            out_e[P * m:P * (m + 1), 512 * n:512 * (n + 1)],
                            ob[:])

    nc.compile()
    _CACHE[key] = nc
    return nc


def _prep_in_maps(q, kv, W_qkv, W_proj, b_proj):
    f16 = np.float16
    q = np.asarray(q, dtype=np.float32)
    kv = np.asarray(kv, dtype=np.float32)
    W_qkv = np.asarray(W_qkv, dtype=np.float32)
    W_proj = np.asarray(W_proj, dtype=np.float32)
    b_proj = np.asarray(b_proj, dtype=np.float32)

    wq = np.ascontiguousarray(W_qkv[:, 0:DIM]).astype(f16)
    wk = np.ascontiguousarray(W_qkv[:, DIM:2 * DIM]).astype(f16)
    wv = np.ascontiguousarray(W_qkv[:, 2 * DIM:3 * DIM]).astype(f16)
    wp = np.ascontiguousarray(W_proj).astype(f16)
    bias = np.ascontiguousarray(b_proj.reshape(1, DIM)).astype(f16)
    kvT = [np.ascontiguousarray(kv[b].T).astype(f16) for b in range(4)]
    qT = [np.ascontiguousarray(q[b].T).astype(f16) for b in range(4)]

    in_maps = []
    for c in range(8):
        b, qh = c // 2, c % 2
        in_maps.append({
            "kvT": kvT[b],
            "qT": np.ascontiguousarray(qT[b][:, NQH * qh:NQH * (qh + 1)]),
            "wq": wq, "wk": wk, "wv": wv, "wp": wp,
            "bias": bias,
        })
    return in_maps


def run(q, kv, W_qkv, W_proj, b_proj, trace=False):
    nc = build()
    in_maps = _prep_in_maps(q, kv, W_qkv, W_proj, b_proj)
    res = run_bass_kernel_spmd(nc, in_maps, list(range(8)), trace=trace)
    out = np.zeros((4, NQ, DIM), np.float32)
    for c in range(8):
        b, qh = c // 2, c % 2
        out[b, NQH * qh:NQH * (qh + 1), :] = res.results[c]["out"]
    return out, res


def kernel(q, kv, W_qkv, W_proj, b_proj):
    out, _ = run(q, kv, W_qkv, W_proj, b_proj)
    return out
